# revision 5
# baseline (speedup 1.0000x reference)
"""Chamfer distance kernel for 8 Trainium2 NeuronCores.

Strategy
--------
pred/target: [B=4, 8192, 3] fp32.  Output: scalar fp32.

Sharding: core c handles batch b = c//2 and pred-row half h = c%2
(4096 pred rows x 8192 target cols).  Each core computes, over its
[4096, 8192] block of the squared-distance matrix:
  - row mins  (pred -> nearest target), complete for its rows
  - col mins  (target -> nearest pred), partial (combined on host
    with the sibling core's mins)

Distance matrix via the GEMM cross-term trick, evaluated ENTIRELY as
bf16 matmuls with full fp32 accuracy: every fp32 operand is split into
3 bf16 terms (8+8+8 mantissa bits >= fp32's 24), and the products
needed are laid out along the contraction (K) dimension:

  dist[n,m] = |p_n|^2 + |t_m|^2 - 2 p.t
            = sum_k L[k,n] * R[k,m]        (K = 24 bf16 rows)

K rows: per coordinate c: (ph,sh) (ph,sm) (pm,sh) (ph,sl) (pm,sm)
(pl,sh) where p = ph+pm+pl (bf16 split of pred coord) and
s = -2*target coord similarly split; plus 3 rows for |p|^2 split
(vs ones) and 3 rows for |t|^2 split.  Dropped cross terms are
O(2^-26) relative -- fp32-level accuracy at bf16 matmul speed
(1 cycle/row instead of fp32's 4).

All transposition/splitting happens on the host in numpy; the device
kernel is pure matmul + min-reduce.  Min-reduction uses two engine
paths so DVE and ACT share the load:
  - direct: DVE tensor_reduce(min) on [128,2048] PSUM groups
  - f16:    ACT copies PSUM->SBUF fp16, DVE folds with 2x-rate
            tensor_tensor(min) ops, final small reduce
"""

import os
import sys

import numpy as np

if "/opt/trn_rl_repo" not in sys.path and os.path.isdir("/opt/trn_rl_repo"):
    sys.path.append("/opt/trn_rl_repo")

import ml_dtypes

import concourse.bacc as bacc
import concourse.mybir as mybir
from concourse import tile
from concourse.bass_utils import run_bass_kernel_spmd

BF16 = ml_dtypes.bfloat16
F32 = np.float32
F64 = np.float64

B = 4
N = 8192  # pred points per batch
M = 8192  # target points per batch
D = 3
CORES = 8
SHARD = N // 2  # pred rows per core (4096)
K = 24  # contraction rows after bf16 splitting

GROUP = 2048  # PSUM group width (4 banks)
MM_N = 512  # moving free dim per matmul (1 PSUM bank fp32)
BIG = 3.0e38  # "+inf" for min identity

# every chunk with (idx % F16_MOD) < F16_NUM uses the ACT+fp16 path
F16_MOD = 4
F16_NUM = 3


def _split3(x64):
    """Split float64 array into 3 bf16 terms summing to ~fp32 accuracy."""
    h = x64.astype(BF16)
    r = x64 - h.astype(F64)
    m = r.astype(BF16)
    r2 = r - m.astype(F64)
    l = r2.astype(BF16)
    return h, m, l


def _cross_rows(a3, b3):
    """Given 3-term splits of two [n,.]-coordinate arrays, return the 6
    row-pairs whose products sum to a*b with O(2^-26) relative error."""
    ah, am, al = a3
    bh, bm, bl = b3
    return [(ah, bh), (ah, bm), (am, bh), (ah, bl), (am, bm), (al, bh)]


def _prep_core(p, t):
    """Host prep for one core: build the 4 [K, n] bf16 operand panels.

    p: [n, 3] f32 pred rows for this core
    t: [m, 3] f32 target rows for this batch
    """
    n, m = p.shape[0], t.shape[0]
    p64 = p.astype(F64)
    t64 = t.astype(F64)
    pn = (p64 * p64).sum(-1)  # [n] |p|^2
    tn = (t64 * t64).sum(-1)  # [m] |t|^2

    ones_n = np.ones(n, BF16)
    ones_m = np.ones(m, BF16)

    psplit = [_split3(p64[:, c]) for c in range(D)]  # pred coords
    ssplit = [_split3(-2.0 * t64[:, c]) for c in range(D)]  # -2*target
    tsplit = [_split3(t64[:, c]) for c in range(D)]  # target coords
    qsplit = [_split3(-2.0 * p64[:, c]) for c in range(D)]  # -2*pred
    pn3 = _split3(pn)
    tn3 = _split3(tn)

    # pass A: out[n_rows, m_cols];  lhsT rows pair with rhs rows
    a_lhs_rows, a_rhs_rows = [], []
    for c in range(D):
        for la, ra in _cross_rows(psplit[c], ssplit[c]):
            a_lhs_rows.append(la)
            a_rhs_rows.append(ra)
    for i in range(3):
        a_lhs_rows.append(pn3[i])
        a_rhs_rows.append(ones_m)
    for i in range(3):
        a_lhs_rows.append(ones_n)
        a_rhs_rows.append(tn3[i])

    # pass B: out[m_rows, n_cols]
    b_lhs_rows, b_rhs_rows = [], []
    for c in range(D):
        for la, ra in _cross_rows(tsplit[c], qsplit[c]):
            b_lhs_rows.append(la)
            b_rhs_rows.append(ra)
    for i in range(3):
        b_lhs_rows.append(tn3[i])
        b_rhs_rows.append(ones_n)
    for i in range(3):
        b_lhs_rows.append(ones_m)
        b_rhs_rows.append(pn3[i])

    return {
        "a_lhs": np.ascontiguousarray(np.stack(a_lhs_rows)),  # [K, n]
        "a_rhs": np.ascontiguousarray(np.stack(a_rhs_rows)),  # [K, m]
        "b_lhs": np.ascontiguousarray(np.stack(b_lhs_rows)),  # [K, m]
        "b_rhs": np.ascontiguousarray(np.stack(b_rhs_rows)),  # [K, n]
    }


def build_in_maps(pred, target, n_rows=SHARD, m_cols=M):
    pred = np.asarray(pred, F32)
    target = np.asarray(target, F32)
    in_maps = []
    for c in range(CORES):
        b, h = divmod(c, 2)
        p = pred[b, h * n_rows : (h + 1) * n_rows]
        t = target[b, :m_cols]
        in_maps.append(_prep_core(p, t))
    return in_maps


def build_nc(n_rows=SHARD, m_cols=M, f16_mod=F16_MOD, f16_num=F16_NUM):
    """Build + compile the per-core Bass program (SPMD across 8 cores)."""
    assert n_rows % GROUP == 0 and m_cols % GROUP == 0
    ca, ga = n_rows // 128, m_cols // GROUP  # pass A: chunks x groups
    cb, gb = m_cols // 128, n_rows // GROUP  # pass B
    qg = GROUP // MM_N  # matmuls per group

    nc = bacc.Bacc()
    dbf = mybir.dt.bfloat16
    df32 = mybir.dt.float32
    df16 = mybir.dt.float16

    a_lhs_d = nc.dram_tensor("a_lhs", [K, n_rows], dbf, kind="ExternalInput")
    a_rhs_d = nc.dram_tensor("a_rhs", [K, m_cols], dbf, kind="ExternalInput")
    b_lhs_d = nc.dram_tensor("b_lhs", [K, m_cols], dbf, kind="ExternalInput")
    b_rhs_d = nc.dram_tensor("b_rhs", [K, n_rows], dbf, kind="ExternalInput")
    out_d = nc.dram_tensor("out", [128, ca + cb], df32, kind="ExternalOutput")

    with tile.TileContext(nc) as tc:
        with (
            tc.tile_pool(name="ops", bufs=1) as ops,
            tc.tile_pool(name="acc", bufs=1) as accp,
            tc.tile_pool(name="psum", bufs=2, space="PSUM") as psum,
            tc.tile_pool(name="f16g", bufs=12) as f16g,
            tc.tile_pool(name="f16t", bufs=8) as f16t,
        ):
            a_lhs = ops.tile([K, n_rows], dbf, tag="a_lhs")
            a_rhs = ops.tile([K, m_cols], dbf, tag="a_rhs")
            b_lhs = ops.tile([K, m_cols], dbf, tag="b_lhs")
            b_rhs = ops.tile([K, n_rows], dbf, tag="b_rhs")
            acc_a = accp.tile([128, ca * ga], df32, tag="acc_a")
            acc_b = accp.tile([128, cb * gb], df32, tag="acc_b")
            d_sb = accp.tile([128, ca + cb], df32, tag="d_sb")

            nc.sync.dma_start(a_lhs[:], a_lhs_d[:])
            nc.sync.dma_start(a_rhs[:], a_rhs_d[:])
            nc.sync.dma_start(b_lhs[:], b_lhs_d[:])
            nc.sync.dma_start(b_rhs[:], b_rhs_d[:])

            nc.vector.memset(acc_a[:], BIG)
            nc.vector.memset(acc_b[:], BIG)

            def do_pass(lhs_sb, rhs_sb, acc, chunks, gpc):
                for ch in range(chunks):
                    lw = lhs_sb[:, ch * 128 : (ch + 1) * 128]
                    use_f16 = (ch % f16_mod) < f16_num
                    gts = []
                    for g in range(gpc):
                        ps = psum.tile([128, GROUP], df32, tag="ps")
                        for q in range(qg):
                            col = g * GROUP + q * MM_N
                            nc.tensor.matmul(
                                ps[:, q * MM_N : (q + 1) * MM_N],
                                lw,
                                rhs_sb[:, col : col + MM_N],
                                start=True,
                                stop=True,
                            )
                        if use_f16:
                            gt = f16g.tile([128, GROUP], df16, tag="gt")
                            nc.scalar.copy(gt[:], ps[:])
                            gts.append(gt)
                        else:
                            nc.vector.tensor_reduce(
                                acc[:, ch * gpc + g : ch * gpc + g + 1],
                                ps[:],
                                axis=mybir.AxisListType.X,
                                op=mybir.AluOpType.min,
                            )
                    if use_f16:
                        # pairwise-fold the group tiles, then halving tree
                        while len(gts) > 1:
                            nxt = []
                            for i in range(0, len(gts) - 1, 2):
                                mg = f16g.tile([128, GROUP], df16, tag="gt")
                                nc.vector.tensor_tensor(
                                    mg[:], gts[i][:], gts[i + 1][:],
                                    op=mybir.AluOpType.min,
                                )
                                nxt.append(mg)
                            if len(gts) % 2:
                                nxt.append(gts[-1])
                            gts = nxt
                        cur = gts[0]
                        sz = GROUP
                        while sz > 128:
                            sz //= 2
                            ft = f16t.tile([128, sz], df16, tag="ft")
                            nc.vector.tensor_tensor(
                                ft[:], cur[:, :sz], cur[:, sz : 2 * sz],
                                op=mybir.AluOpType.min,
                            )
                            cur = ft
                        nc.vector.tensor_reduce(
                            acc[:, ch * gpc : ch * gpc + 1],
                            cur[:],
                            axis=mybir.AxisListType.X,
                            op=mybir.AluOpType.min,
                        )

            do_pass(a_lhs, a_rhs, acc_a, ca, ga)
            do_pass(b_lhs, b_rhs, acc_b, cb, gb)

            nc.vector.tensor_reduce(
                d_sb[:, 0:ca],
                acc_a[:].rearrange("p (c g) -> p c g", g=ga),
                axis=mybir.AxisListType.X, op=mybir.AluOpType.min,
            )
            nc.vector.tensor_reduce(
                d_sb[:, ca : ca + cb],
                acc_b[:].rearrange("p (c g) -> p c g", g=gb),
                axis=mybir.AxisListType.X, op=mybir.AluOpType.min,
            )
            nc.sync.dma_start(out_d[:], d_sb[:])

    nc.compile()
    return nc


def combine(outs, n_rows=SHARD, m_cols=M):
    """Host combine: outs = list of 8 [128, ca+cb] arrays -> scalar."""
    ca = n_rows // 128
    cb = m_cols // 128
    d_pt = []  # every pred row's min, each appears exactly once
    d_tp = []  # per batch: elementwise min of the two sibling cores
    for b in range(B):
        o0 = outs[2 * b].astype(F64)
        o1 = outs[2 * b + 1].astype(F64)
        d_pt.append(o0[:, :ca])
        d_pt.append(o1[:, :ca])
        d_tp.append(np.minimum(o0[:, ca : ca + cb], o1[:, ca : ca + cb]))
    mean_pt = np.concatenate([x.ravel() for x in d_pt]).mean()
    mean_tp = np.concatenate([x.ravel() for x in d_tp]).mean()
    return np.float32(mean_pt + mean_tp)


_NC_CACHE = {}


def kernel(pred, target):
    key = (SHARD, M)
    if key not in _NC_CACHE:
        _NC_CACHE[key] = build_nc()
    nc = _NC_CACHE[key]
    in_maps = build_in_maps(pred, target)
    res = run_bass_kernel_spmd(nc, in_maps, core_ids=list(range(CORES)))
    outs = [res.results[c]["out"] for c in range(CORES)]
    return combine(outs)
